# revision 52
# baseline (speedup 1.0000x reference)
"""Causal self-attention with RoPE on 8 Trainium2 NeuronCores.

Sharding: batch x head-group. Core c handles batch b = c//2 and head group
g = c%2 (8 of 16 heads). Each core runs the full per-(batch, head-group)
pipeline on device; the host sums the two partial output projections per
batch and adds b_out.

v2 layout (chunk-pipelined for PE warmth):
  The TRN2 PE clock-gates to 1.2 GHz after any ~3.4us idle window and only
  reaches 2.4 GHz under sustained work, so the whole kernel is emitted as a
  single software-pipelined stream: QKV projection for chunk c+1 and the
  output projection for chunk c are "filler" PE work that the Tile list
  scheduler pulls into the gaps of chunk c's attention (which is paced by
  ACT exp). PSUM budget (8 banks): 2 x [128,1024] score tiles + 2 PV
  accumulators + 2 filler tiles.

  - Scores for the two heads of a pair go into one [128,1024] PSUM tile
    (two banks); their K=64 matmuls land in disjoint PE row groups
    (tile_position auto-derives from lhsT base partition) so they can
    overlap on the array. One [128,1024] exp per (pr, kt) on ACT.
  - Causal mask: multiplicative 0/1 bf16 mask on the exp tile (diagonal
    128-tiles only), broadcast across the two heads in one DVE op.
  - PV matmul: V gets a ones column (M=65) so row 64 of the PV psum
    accumulates the softmax denominator for free.
  - Epilogue per (chunk, pair): DMA the two denominator rows out of PSUM,
    one reciprocal_approx_fast [2,512], DMA-broadcast each row to 64
    partitions, two DVE multiplies into yT (the upper-head half staged
    through a base-0 temp + DMA because elementwise ops cannot change
    partition base).
  - QKV bias rides DVE tensor_scalar_add (PSUM->SBUF cast+bias in one op)
    so ACT does nothing but exp.
"""

import os
import sys

if "/opt/trn_rl_repo" not in sys.path:
    sys.path.insert(0, "/opt/trn_rl_repo")

import numpy as np
import ml_dtypes

import concourse.bass as bass
import concourse.mybir as mybir
import concourse.tile as tile

F32 = mybir.dt.float32
BF16 = mybir.dt.bfloat16

B, L, D = 4, 2048, 1024
H, DH = 16, 64
NCORES = 8
G = 2                 # head groups (cores per batch)
HPC = H // G          # heads per core = 8
DQ = HPC * DH         # per-core q/k/v width = 512
PAIRS = HPC // 2      # 128-partition head pairs = 4
CHUNK = 512           # query-chunk (matmul free dim)
NCH = L // CHUNK      # 4
KT = D // 128         # 8 k-tiles over d_model
LT = L // 128         # 16 l-tiles
VW = DH + 1           # V columns per head incl. ones column = 65

LAST_RESULTS = None   # test harness reads perf fields from here


def legalize_bir_waits(bir_json: bytes) -> bytes:
    """Split multi-wait sync_infos into standalone EventSemaphore instrs.

    This container's walrus codegen accepts at most ONE sync wait per
    instruction (two for EventSemaphore), but Tile's sem assigner happily
    attaches several.  For every instruction carrying N>1 waits, keep one
    and hoist the rest onto EventSemaphore instructions inserted directly
    before it on the same engine (same block), which preserves each
    engine's program order and therefore the sync semantics.
    """
    import json as _json

    j = _json.loads(bir_json)
    uid = [0]
    for fn in j["functions"]:
        for blk in fn["blocks"]:
            out_insts = []
            for inst in blk["instructions"]:
                si = inst.get("sync_info")
                waits = (si or {}).get("on_wait") or []
                cap = 2 if inst.get("opcode") == "EventSemaphore" else 1
                if len(waits) > cap:
                    extra, keep = waits[:-cap], waits[-cap:]
                    for i in range(0, len(extra), 2):
                        uid[0] += 1
                        out_insts.append(
                            {
                                "name": f"antwaitfix-{uid[0]}",
                                "opcode": "EventSemaphore",
                                "engine": inst["engine"],
                                "ins": [],
                                "outs": [],
                                "debug": inst.get("debug", 0),
                                "sync_info": {
                                    "on_wait": extra[i : i + 2],
                                    "on_update": [],
                                },
                            }
                        )
                    si["on_wait"] = keep
                out_insts.append(inst)
            blk["instructions"] = out_insts
    return _json.dumps(j).encode()


def build_module():
    nc = bass.Bass(use_seq_codegen=True)

    xT = nc.declare_dram_parameter("xT", [D, L], BF16, isOutput=False)
    wq = nc.declare_dram_parameter("wq", [D, DQ], BF16, isOutput=False)
    wk = nc.declare_dram_parameter("wk", [D, DQ], BF16, isOutput=False)
    wv = nc.declare_dram_parameter("wv", [D, DQ], BF16, isOutput=False)
    wo = nc.declare_dram_parameter("wo", [DQ, D], BF16, isOutput=False)
    bq = nc.declare_dram_parameter("bq", [128, PAIRS], F32, isOutput=False)
    bk = nc.declare_dram_parameter("bk", [128, PAIRS], F32, isOutput=False)
    bv = nc.declare_dram_parameter("bv", [128, DQ], F32, isOutput=False)
    cosT = nc.declare_dram_parameter("cosT", [128, L], BF16, isOutput=False)
    sinT = nc.declare_dram_parameter("sinT", [128, L], BF16, isOutput=False)
    maskb = nc.declare_dram_parameter("maskb", [128, 896], BF16, isOutput=False)
    sperm = nc.declare_dram_parameter("sperm", [128, 128], BF16, isOutput=False)
    out = nc.declare_dram_parameter("out", [L, D], F32, isOutput=True)

    with tile.TileContext(nc) as tc:
        with (
            tc.tile_pool(name="const", bufs=1) as cp,
            tc.tile_pool(name="acts", bufs=1) as ap,
            tc.tile_pool(name="work", bufs=4) as wp,
            tc.tile_pool(name="sc", bufs=2, space="PSUM") as scp,
            tc.tile_pool(name="pv", bufs=2, space="PSUM") as pvp,
            tc.tile_pool(name="fp", bufs=2, space="PSUM") as fpp,
        ):
            # ---- input loads. The SP sequencer spends ~565ns configuring
            # each dma_start, so loads are issued as ONE config per tensor
            # (the descriptors still spread across DMA engines); the small
            # constants ride the ACT sequencer, which is idle until the
            # first exp. xT's first column-chunk is its own config so chunk-0
            # projection isn't gated on the full 4MB activation load.
            xT_sb = ap.tile([128, KT, L], BF16)
            wq_sb = cp.tile([128, KT, DQ], BF16)
            wk_sb = cp.tile([128, KT, DQ], BF16)
            wv_sb = cp.tile([128, KT, DQ], BF16)
            xTr = xT.rearrange("(kt p) l -> p kt l", p=128)
            wqr = wq.rearrange("(kt p) m -> p kt m", p=128)
            wkr = wk.rearrange("(kt p) m -> p kt m", p=128)
            wvr = wv.rearrange("(kt p) m -> p kt m", p=128)
            # first-needed tensors split in half across the SP and ACT
            # sequencers (config time ~300ns + ~3ns/descriptor each)
            nc.sync.dma_start(wq_sb[:, 0:2, :], wqr[:, 0:2, :])
            nc.scalar.dma_start(wq_sb[:, 4:6, :], wqr[:, 4:6, :])
            nc.sync.dma_start(wq_sb[:, 2:4, :], wqr[:, 2:4, :])
            nc.scalar.dma_start(wq_sb[:, 6:8, :], wqr[:, 6:8, :])
            nc.sync.dma_start(xT_sb[:, 0:2, 0:CHUNK], xTr[:, 0:2, 0:CHUNK])
            nc.scalar.dma_start(xT_sb[:, 4:6, 0:CHUNK], xTr[:, 4:6, 0:CHUNK])
            nc.sync.dma_start(xT_sb[:, 2:4, 0:CHUNK], xTr[:, 2:4, 0:CHUNK])
            nc.scalar.dma_start(xT_sb[:, 6:8, 0:CHUNK], xTr[:, 6:8, 0:CHUNK])
            bq_sb = cp.tile([128, PAIRS], F32)
            bk_sb = cp.tile([128, PAIRS], F32)
            bv_sb = cp.tile([128, DQ], F32)
            cos_sb = cp.tile([128, L], BF16)
            sin_sb = cp.tile([128, L], BF16)
            mask_sb = cp.tile([128, 896], BF16)
            sperm_sb = cp.tile([128, 128], BF16)
            nc.sync.dma_start(wk_sb[:, 0:4, :], wkr[:, 0:4, :])
            nc.scalar.dma_start(wk_sb[:, 4:8, :], wkr[:, 4:8, :])
            nc.scalar.dma_start(bq_sb[:], bq[:])
            nc.scalar.dma_start(bk_sb[:], bk[:])
            nc.scalar.dma_start(cos_sb[:], cosT[:])
            nc.scalar.dma_start(sin_sb[:], sinT[:])
            nc.scalar.dma_start(sperm_sb[:], sperm[:])
            nc.sync.dma_start(wv_sb[:, 0:4, :], wvr[:, 0:4, :])
            nc.scalar.dma_start(wv_sb[:, 4:8, :], wvr[:, 4:8, :])
            nc.scalar.dma_start(bv_sb[:], bv[:])
            nc.scalar.dma_start(mask_sb[:], maskb[:])
            nc.sync.dma_start(
                xT_sb[:, :, CHUNK:L], xTr[:, :, CHUNK:L]
            )
            wo_sb = cp.tile([128, PAIRS, D], BF16)
            nc.sync.dma_start(
                wo_sb[:], wo.rearrange("(pr p) c -> p pr c", p=128)
            )

            # Selector rows for the denominator-broadcast matmuls:
            # sel[:, 0, :] = [1]*64 + [0]*64, sel[:, 1, :] = its complement.
            sel_sb = cp.tile([128, 2, 128], BF16)
            nc.vector.memset(sel_sb[:, 0, 0:64], 1.0)
            nc.vector.memset(sel_sb[:, 0, 64:128], 0.0)
            nc.vector.memset(sel_sb[:, 1, 0:64], 0.0)
            nc.vector.memset(sel_sb[:, 1, 64:128], 1.0)

            # ~5us of zero-dependency garbage matmuls at t=0: the PE clock
            # gate (HAM) needs ~3.4us of sustained activity to release the
            # 1.2GHz throttle, and the real first matmuls are DMA-gated
            # until ~5us — so the prologue would otherwise run at half clock.
            wu_ps = fpp.tile([128, CHUNK], F32, tag="fp", name="warmup")
            for i in range(16):
                nc.tensor.matmul(
                    wu_ps[:, 0:256],
                    sel_sb[:, 0, :],
                    sel_sb.rearrange("p a b -> p (a b)")[:, 0:256],
                    start=True,
                    stop=True,
                )

            qT_sb = ap.tile([128, PAIRS, L], BF16)
            kT_sb = ap.tile([128, PAIRS, L], BF16)
            v_sb = ap.tile([128, LT, HPC * VW], BF16)
            yT_sb = ap.tile([128, PAIRS, L], BF16)
            # ones columns of V, set once for all l-tiles
            v4 = v_sb.rearrange("p lt (h c) -> p lt h c", c=VW)
            nc.vector.memset(v4[:, :, :, DH:VW], 1.0)

            def qk_group(c, mt, which):
                cs = slice(c * CHUNK, (c + 1) * CHUNK)
                dst, w_sb, b_sb = (
                    (qT_sb, wq_sb, bq_sb) if which == "q" else (kT_sb, wk_sb, bk_sb)
                )
                ps = fpp.tile(
                    [128, CHUNK], F32, tag="fp", name=f"{which}_{c}_{mt}"
                )
                for kt in range(KT):
                    nc.tensor.matmul(
                        ps[:],
                        w_sb[:, kt, mt * 128 : (mt + 1) * 128],
                        xT_sb[:, kt, cs],
                        start=(kt == 0),
                        stop=(kt == KT - 1),
                    )
                nc.vector.tensor_scalar_add(
                    dst[:, mt, cs], ps[:], b_sb[:, mt : mt + 1]
                )

            def v_group(lt):
                ps = fpp.tile([128, CHUNK], F32, tag="fp", name=f"v_{lt}")
                for kt in range(KT):
                    nc.tensor.matmul(
                        ps[:],
                        xT_sb[:, kt, lt * 128 : (lt + 1) * 128],
                        wv_sb[:, kt, :],
                        start=(kt == 0),
                        stop=(kt == KT - 1),
                    )
                vdst = v_sb[:, lt, :].rearrange("p (h c) -> p h c", c=VW)
                nc.vector.tensor_add(vdst[:, :, 0:DH], ps[:], bv_sb[:])

            def rope_group(c, mt):
                # rotate_half's partition swap (with sign) rides the PE as a
                # signed-permutation matmul — a DMA-based swap costs 4 SP
                # sequencer configs per tile, which starves the DMA pipeline.
                cs = slice(c * CHUNK, (c + 1) * CHUNK)
                for dst in (qT_sb, kT_sb):
                    t = dst[:, mt, cs]
                    sw_ps = fpp.tile([128, CHUNK], F32, tag="fp",
                                     name=f"sw_{c}_{mt}")
                    nc.tensor.matmul(
                        sw_ps[:], sperm_sb[:], t, start=True, stop=True
                    )
                    swp = wp.tile([128, CHUNK], BF16, tag="swp",
                                  name=f"swp_{c}_{mt}")
                    nc.vector.tensor_mul(swp[:], sw_ps[:], sin_sb[:, cs])
                    nc.vector.tensor_mul(t, t, cos_sb[:, cs])
                    nc.vector.tensor_add(t, t, swp[:])

            def proj_closures(c):
                fs = []
                for mt in range(PAIRS):
                    fs.append(lambda c=c, mt=mt: qk_group(c, mt, "q"))
                    fs.append(lambda c=c, mt=mt: qk_group(c, mt, "k"))
                    fs.append(lambda c=c, mt=mt: rope_group(c, mt))
                    if mt == 0:
                        for lt in range(4 * c, 4 * c + 4):
                            fs.append(lambda lt=lt: v_group(lt))
                return fs

            def outproj_group(lt, cc):
                ps = fpp.tile([128, CHUNK], F32, tag="fp", name=f"op_{lt}_{cc}")
                for pr in range(PAIRS):
                    nc.tensor.matmul(
                        ps[:],
                        yT_sb[:, pr, lt * 128 : (lt + 1) * 128],
                        wo_sb[:, pr, cc * CHUNK : (cc + 1) * CHUNK],
                        start=(pr == 0),
                        stop=(pr == PAIRS - 1),
                    )
                ob = wp.tile([128, CHUNK], F32, tag="ob", name=f"ob_{lt}_{cc}")
                nc.scalar.copy(ob[:], ps[:])
                nc.sync.dma_start(
                    out[lt * 128 : (lt + 1) * 128, cc * CHUNK : (cc + 1) * CHUNK],
                    ob[:],
                )

            def outproj_closures(c):
                return [
                    lambda lt=lt, cc=cc: outproj_group(lt, cc)
                    for lt in range(4 * c, 4 * c + 4)
                    for cc in range(2)
                ]

            def attn_scores(c, pr, kt):
                # columns col < k0-q0 are fully causal-masked (every key in
                # this tile sits above the query), so scores/exp/mask/PV all
                # run on the shrunk column range [off, CHUNK).
                q0 = c * CHUNK
                k0 = kt * 128
                off = max(0, k0 - q0)
                sct = scp.tile(
                    [128, 2, CHUNK], F32, tag="sc", name=f"sc_{c}_{pr}_{kt}"
                )
                for hh in range(2):
                    nc.tensor.matmul(
                        sct[:, hh, off:CHUNK],
                        kT_sb[hh * 64 : (hh + 1) * 64, pr, k0 : k0 + 128],
                        qT_sb[hh * 64 : (hh + 1) * 64, pr, q0 + off : q0 + CHUNK],
                        start=True,
                        stop=True,
                    )
                ex = wp.tile(
                    [128, 2, CHUNK], BF16, tag="ex", bufs=8,
                    name=f"ex_{c}_{pr}_{kt}"
                )
                nc.scalar.activation(
                    ex[:, :, off:CHUNK],
                    sct[:, :, off:CHUNK],
                    mybir.ActivationFunctionType.Exp,
                    scale=float(1.0 / np.sqrt(DH)),
                )
                if k0 >= q0:
                    s = 384 - (k0 - q0)
                    mbc = (
                        mask_sb[:, s + off : s + CHUNK]
                        .unsqueeze(1)
                        .broadcast_to([128, 2, CHUNK - off])
                    )
                    nc.vector.tensor_mul(
                        ex[:, :, off:CHUNK], ex[:, :, off:CHUNK], mbc
                    )
                return ex

            def attn_pv(c, pr, kt, ys, ex, n_lk):
                q0 = c * CHUNK
                off = max(0, kt * 128 - q0)
                for hh in range(2):
                    h = 2 * pr + hh
                    nc.tensor.matmul(
                        ys[hh][0:VW, off:CHUNK],
                        v_sb[:, kt, h * VW : (h + 1) * VW],
                        ex[:, hh, off:CHUNK],
                        start=(kt == 0),
                        stop=(kt == n_lk - 1),
                    )

            def attn_epilogue(c, pr, ys):
                # normalize by the denominator row (row 64). PSUM can't feed
                # DMA or matmul-rhs, so: DVE copies the raw denominator rows
                # PSUM->SBUF (same base partition 64), a DMA bounces them to
                # DRAM and back with a stride-0 read that replicates them to
                # 64 partitions, and ACT computes 1/x as exp(-ln(x)) (both
                # funcs share one table set; DVE reciprocal costs 6.5ns/elem,
                # there is no DVE divide, and the custom-DVE approx ops don't
                # survive this walrus).
                q0 = c * CHUNK
                den_r = wp.tile(
                    [128, 2, CHUNK], BF16, tag="denr", bufs=2,
                    name=f"denr_{c}_{pr}",
                )
                for hh in range(2):
                    nc.vector.tensor_copy(den_r[64:65, hh, :], ys[hh][64:65, :])
                bc_ps = fpp.tile([128, CHUNK], F32, tag="fp", name=f"bc_{c}_{pr}")
                for hh in range(2):
                    nc.tensor.matmul(
                        bc_ps[:],
                        sel_sb[64:65, hh, :],
                        den_r[64:65, hh, :],
                        start=(hh == 0),
                        stop=(hh == 1),
                    )
                lnb = wp.tile([128, CHUNK], F32, tag="lnb", bufs=2,
                              name=f"lnb_{c}_{pr}")
                nc.scalar.activation(
                    lnb[:], bc_ps[:], mybir.ActivationFunctionType.Ln
                )
                bcs = wp.tile([128, CHUNK], F32, tag="bcs", bufs=2,
                              name=f"bcs_{c}_{pr}")
                nc.scalar.activation(
                    bcs[:], lnb[:], mybir.ActivationFunctionType.Exp,
                    scale=-1.0,
                )
                bcs1 = wp.tile([64, CHUNK], F32, tag="bcs1", bufs=2,
                               name=f"bcs1_{c}_{pr}")
                nc.sync.dma_start(bcs1[:], bcs[64:128, :])
                nc.vector.tensor_mul(
                    yT_sb[0:64, pr, q0 : q0 + CHUNK], ys[0][0:64, :], bcs[0:64, :]
                )
                yt = wp.tile([64, CHUNK], BF16, tag="yt", name=f"yt_{c}_{pr}")
                nc.vector.tensor_mul(yt[:], ys[1][0:64, :], bcs1[:])
                nc.sync.dma_start(yT_sb[64:128, pr, q0 : q0 + CHUNK], yt[:])

            def attn_chunk(c, fillers):
                """Emit chunk c's attention iterations with filler groups
                interleaved evenly (priority-spreading: the list scheduler
                prefers earlier-emitted work, so clustering fillers starves
                ACT of exp work while PE grinds through them)."""
                n_lk = 4 * (c + 1)
                n_iters = PAIRS * n_lk
                pace = len(fillers) / max(n_iters, 1)
                credit = 0.0
                for pr in range(PAIRS):
                    ys = [
                        pvp.tile(
                            [128, CHUNK], F32, tag="pv", name=f"ys_{c}_{pr}_{hh}"
                        )
                        for hh in range(2)
                    ]
                    # software-pipelined: PV(kt-1) is emitted BEFORE the
                    # scores of kt, so the PV pair issues the moment its exp
                    # lands and the score pair streams back-to-back behind it
                    prev_ex = None
                    for kt in range(n_lk):
                        if prev_ex is not None:
                            attn_pv(c, pr, kt - 1, ys, prev_ex, n_lk)
                        prev_ex = attn_scores(c, pr, kt)
                        credit += pace
                        while credit >= 1.0 and fillers:
                            fillers.popleft()()
                            credit -= 1.0
                    attn_pv(c, pr, n_lk - 1, ys, prev_ex, n_lk)
                    attn_epilogue(c, pr, ys)
                while fillers:
                    fillers.popleft()()

            from collections import deque

            p0 = proj_closures(0)
            # startup: emit only what attention(0, pr=0, kt=0) needs (q0/k0/
            # rope0 + the first v tile); everything else is interleaved
            # filler. outproj(1) is deferred to chunk 3, which has 64
            # attention iterations but almost no projection work left.
            for f in p0[:4]:
                f()
            attn_chunk(0, deque(p0[4:] + proj_closures(1)))
            attn_chunk(1, deque(proj_closures(2)))
            attn_chunk(2, deque(proj_closures(3)))
            attn_chunk(3, deque(outproj_closures(0) + outproj_closures(1)
                                + outproj_closures(2)))
            for f in outproj_closures(3):
                f()
    return nc


def _rope_tables():
    inv_freq = (1.0 / (10000.0 ** (np.arange(0, DH, 2, dtype=np.float32) / DH))).astype(
        np.float32
    )
    t = np.arange(L, dtype=np.float32)
    freqs = np.einsum("l,d->ld", t, inv_freq).astype(np.float32)  # (L, 32)
    emb = np.concatenate([freqs, freqs], axis=-1)                 # (L, 64)
    cos = np.cos(emb).astype(np.float32)
    sin = np.sin(emb).astype(np.float32)
    # rotate_half's sign is folded into the sperm permutation matrix
    cos128 = np.tile(cos.T, (2, 1))                # (128, L)
    sin128 = np.tile(sin.T, (2, 1))
    return cos128, sin128


def _sperm():
    # swp = sperm.T @ q implements rotate_half per 64-row head block:
    # swp[m] = -q[m+32] for m in [0,32), swp[m] = +q[m-32] for m in [32,64)
    S = np.zeros((128, 128), np.float32)
    for h in range(2):
        b = 64 * h
        for m in range(32):
            S[b + m + 32, b + m] = -1.0
            S[b + m, b + m + 32] = 1.0
    return S


def _mask_big():
    # maskb[p, j] = 1.0 iff p <= j - 384 (slice at s = 384-delta gives the
    # diagonal-tile mask "p <= f - delta")
    p = np.arange(128)[:, None]
    j = np.arange(896)[None, :]
    return (p <= j - 384).astype(np.float32)


def _bf16(a):
    return np.asarray(a, dtype=np.float32).astype(ml_dtypes.bfloat16)


_COMPILED = None


def _ensure_trace_hook() -> bool:
    """Install the axon NTFF profile hook if the boot shim couldn't.

    The image's `antenv` stub lacks `axon_hooks`, so bass_utils' trace
    path crashes on import. Synthesize the module and wire in the ctypes
    hook from trn_agent_boot. Returns True iff tracing is usable.
    """
    try:
        from antenv.axon_hooks import get_axon_ntff_profile_hook  # noqa: F401

        return True
    except ImportError:
        pass
    try:
        import types

        import antenv
        import trn_agent_boot.trn_boot as tb

        mod = types.ModuleType("antenv.axon_hooks")
        _hook = [None]
        mod.set_axon_ntff_profile_hook = lambda h: _hook.__setitem__(0, h)
        mod.get_axon_ntff_profile_hook = lambda: _hook[0]
        sys.modules["antenv.axon_hooks"] = mod
        antenv.axon_hooks = mod
        mod.set_axon_ntff_profile_hook(
            tb._ntff_profile_via_ctypes("/opt/axon/libaxon_pjrt.so")
        )
        return True
    except Exception:
        return False


def kernel(x, pad_mask, W_qkv, b_qkv, W_out, b_out):
    global LAST_RESULTS, _COMPILED
    from concourse.bass_utils import run_bass_kernel_spmd

    x = np.asarray(x, dtype=np.float32)
    W_qkv = np.asarray(W_qkv, dtype=np.float32)
    b_qkv = np.asarray(b_qkv, dtype=np.float32)
    W_out = np.asarray(W_out, dtype=np.float32)
    b_out = np.asarray(b_out, dtype=np.float32)

    cos128, sin128 = _rope_tables()
    maskb = _mask_big()

    in_maps = []
    for core in range(NCORES):
        b, g = core // G, core % G
        sl = slice(g * DQ, (g + 1) * DQ)
        wq = W_qkv[:, 0 * D : 1 * D][:, sl]
        wk = W_qkv[:, 1 * D : 2 * D][:, sl]
        wv = W_qkv[:, 2 * D : 3 * D][:, sl]
        bqv = b_qkv[0 * D : 1 * D][sl]
        bkv = b_qkv[1 * D : 2 * D][sl]
        bvv = b_qkv[2 * D : 3 * D][sl]
        in_maps.append(
            {
                "xT": _bf16(x[b].T),
                "wq": _bf16(wq),
                "wk": _bf16(wk),
                "wv": _bf16(wv),
                "wo": _bf16(W_out[sl, :]),
                "bq": np.ascontiguousarray(bqv.reshape(PAIRS, 128).T),
                "bk": np.ascontiguousarray(bkv.reshape(PAIRS, 128).T),
                "bv": np.tile(bvv[None, :], (128, 1)).astype(np.float32),
                "cosT": _bf16(cos128),
                "sinT": _bf16(sin128),
                "maskb": _bf16(maskb),
                "sperm": _bf16(_sperm()),
            }
        )

    if _COMPILED is None:
        nc = build_module()
        fixed = legalize_bir_waits(nc.to_json_bytes())
        nc.to_json_bytes = lambda: fixed  # bass2jax ships this BIR to walrus
        _COMPILED = nc
    nc = _COMPILED

    res = run_bass_kernel_spmd(
        nc,
        in_maps,
        core_ids=list(range(NCORES)),
        trace=bool(os.environ.get("BASS_TRACE")) and _ensure_trace_hook(),
    )
    LAST_RESULTS = res

    out = np.zeros((B, L, D), dtype=np.float32)
    for core in range(NCORES):
        out[core // G] += np.asarray(res.results[core]["out"], dtype=np.float32)
    out += b_out[None, None, :]
    return out
